# revision 30
# baseline (speedup 1.0000x reference)
"""EnhancedDNCMemory forward step on 8 Trainium2 NeuronCores.

Strategy
--------
The only heavy tensor is the temporal link matrix ``link`` [B=4, N=4096,
N=4096]. Everything else is O(N) or O(N*W) and is computed on the host in
float32.

The reference computes (per batch)::

    link_new = (1 - w_i - w_j) * link + w_i * p_j          (diag zeroed)
    fwd[r]   = link_new   @ rwp[r]
    bwd[r]   = link_new^T @ rwp[r]

Expanding link_new, both contractions decompose into matvecs against the
*raw* link matrix with the 8 stacked vectors V = [rwp^T | (w*rwp)^T] (N x 8):

    fwd[r,i] = (1-w_i)*(L@rwp_r)_i - (L@(w*rwp_r))_i + w_i*(p.rwp_r) - w_i*p_i*rwp_r_i
    bwd[r,i] = (1-w_i)*(L^T@rwp_r)_i - (L^T@(w*rwp_r))_i + p_i*(w.rwp_r) - w_i*p_i*rwp_r_i

So the device only computes Y1 = L_slab @ V and Y2 = L_slab^T @ V_slab.
Sharding: 8 cores = 4 batches x 2 row-slabs of 2048 rows. Y1 slabs
concatenate; Y2 partials sum (both on host, they are [N, 8] per core).

Device kernel (fp8 + DoubleRow + partial on-chip transposition):
  - Everything streams in fp8 e4m3 with an exact x4096 power-of-2 host
    prescale (values are O(1/N)); f32 PSUM accumulation. The 2^24 output
    scale divides out exactly. End-to-end relative error ~2e-4 (quantization
    errors average out across the 4096-term sums).
  - Y2 = L^T V streams the native-layout slab (contraction over rows =
    partitions) through MatmulPerfMode.DoubleRow matmuls: both operands are
    fp8 tiles shaped [128, 2, F], so each PE column-cycle contracts 256
    elements -- 2x the bf16 ingestion rate.
  - Y1 = L V needs the transposed orientation. For the ALPHA stripes
    (j outside TBETA) the host uploads a pre-transposed fp8 copy, consumed
    by the same DoubleRow form. For the BETA stripes the device reuses the
    native tiles: one DVE 32x32 block-transpose per stripe (8.7 us each,
    hidden under the stream) plus 64 row-grouped K=32 matmuls per stripe
    (4 concurrent via tile_position), accumulated in 4 PSUM tiles with a
    strided i-map that the host reorders. This trades ~3.4 us of spare PE
    time per stripe for 1.05 MB of HBM traffic.
  - DMA: ~14.7 MB/core in ~2 MB transfers alternating whole transfers
    across both HWDGE rings (descriptor generation is ~2.6 us per dma_start
    regardless of size -- one ring alone cannot feed 16 SDMA engines), with
    Y1/Y2 work interleaved so tile arrivals match the PE's static order.
    Stores and the rowgroup consts ride the separate SWDGE (gpsimd) path.
  - Both HBM copies are host-preswizzled so every DMA reads contiguous
    per-partition lines.

Toolchain notes: walrus on this stack allows at most ONE sync-wait per
instruction -- _legalize_waits() drops redundant same-engine waits and hoists
the rest onto same-engine NoOps, and the Tile kernel-tail drain is split
into one Drain per outstanding semaphore.
"""

import os

import ml_dtypes
import numpy as np

B = 4
N = 4096
W = 64
R = 4
NCORES = 8
SLAB = N // 2  # rows per core
EPS = 1e-6

NJT = N // 512  # 8 column tiles (Y2)
NIB = SLAB // 512  # 4 output row blocks (Y1)
TBETA = (6, 7)  # stripes whose Y1 j-contraction runs on-chip (DVE + rowgroups)
NBETA = len(TBETA)
KARR = tuple(
    k for t in range(NJT) if t not in TBETA for k in (2 * t, 2 * t + 1)
)  # tmat j-chunks (alpha stripes)
NKA = len(KARR)

_NC = None
LAST_RESULT = None


# ----------------------------------------------------------------- device ---


def _build_program():
    import concourse.bass as bass
    import concourse.mybir as mybir
    from concourse.tile import TileContext

    F8 = mybir.dt.float8e4
    F32 = mybir.dt.float32
    DR = mybir.MatmulPerfMode.DoubleRow

    class SplitDrainTileContext(TileContext):
        """Split the kernel-tail drain: walrus caps sync-waits per inst at 1."""

        def _drain_and_barrier(self, tick_clock, wait_clock):
            from concourse.vector_clock import ScopedClock, VectorClock

            vec = list(tick_clock.global_clock)
            nz = [i for i, t in enumerate(vec) if t > 0]
            for proc in nz:
                pv = VectorClock(
                    [t if j == proc else 0 for j, t in enumerate(vec)]
                )
                d = self.nc.sync.drain()
                wait_clock.add_sem_waits(d.ins, ScopedClock({None: pv}))
            if not nz:
                d = self.nc.sync.drain()
                wait_clock.add_sem_waits(
                    d.ins, ScopedClock({None: tick_clock.global_clock})
                )
            self.nc.all_engine_barrier()
            assert self.sems is not None
            popped = self.nc._tile_sem_poison_stack.pop()
            assert popped is self._sem_poison
            self.nc.clear_and_free_semaphores(list(self.sems.allocated().values()))
            self.nc.all_engine_barrier()

    nc = bass.Bass()
    # native slab, preswizzled: [p, t*8192 + k*1024 + m*512 + n]
    #   = L[r0 + 256k + 128m + p, 512t + n]
    lmat = nc.dram_tensor("lmat", [128, NJT * 8192], F8, kind="ExternalInput")
    # transposed slab, ALPHA stripes only: [p, v, k', m, n]
    #   = L[r0 + 512v + n, 256*KARR[k'] + 128m + p]
    tmat = nc.dram_tensor("tmat", [128, NIB * NKA * 1024], F8, kind="ExternalInput")
    # DR V chunks [p, q, m, 16] (q<8: VS, q=8+k': VF[KARR[k']]) + rowgroup V
    # [p=32a+u, tb, b, c8] = VF[512*TBETA[tb] + 32b + u, c]
    # (c padded 8 -> 16: dual-fp8 LDWEIGHTS requires the Ko=2 interleave
    # step to be a multiple of 16 bytes -- walrus s3_lw_dual_fp8_restrictions)
    NC1 = (8 + NKA) * 32
    consts = nc.dram_tensor(
        "consts", [128, NC1 + NBETA * 128], F8, kind="ExternalInput"
    )
    y1t = nc.dram_tensor("y1t", [8, SLAB], F32, kind="ExternalOutput")
    y1bt = nc.dram_tensor("y1bt", [32, 512], F32, kind="ExternalOutput")
    y2t = nc.dram_tensor("y2t", [8, N], F32, kind="ExternalOutput")

    lv = lmat[:, :].rearrange("p (t k m n) -> p t k m n", t=NJT, k=8, m=2, n=512)
    tv = tmat[:, :].rearrange("p (v k m n) -> p v k m n", v=NIB, k=NKA, m=2, n=512)
    cv = consts[:, 0:NC1].rearrange("p (q m c) -> p q m c", m=2, c=16)
    cv2 = consts[:, NC1:].rearrange("p (tb b c) -> p tb b c", b=16, c=8)

    with SplitDrainTileContext(nc) as tc:
        with (
            tc.tile_pool(name="cpool", bufs=1) as cpool,
            tc.tile_pool(name="l2pool", bufs=3) as l2pool,
            tc.tile_pool(name="l1pool", bufs=3) as l1pool,
            tc.tile_pool(name="apool", bufs=2) as apool,
            tc.tile_pool(name="spool", bufs=3) as spool,
            tc.tile_pool(name="ybpool", bufs=1, space="PSUM") as ybpool,
            tc.tile_pool(name="y1pool", bufs=2, space="PSUM") as y1pool,
            tc.tile_pool(name="y2pool", bufs=2, space="PSUM") as y2pool,
        ):
            # HWDGE descriptor generation costs ~2.6 us per dma_start
            # (always ~128 partition descriptors, size-independent), so the
            # stream uses FEW LARGE transfers alternating whole transfers
            # across the two rings (SP via nc.sync, ACT via nc.scalar) for
            # parallel generation at ~420 GB/s aggregate. Y1 and Y2 work is
            # INTERLEAVED so the PE's static order matches the arrival
            # stream (no dead zones from pool recycling); the stream starts
            # and ends with 1 MiB singles (fast ramp, short PE tail) and
            # both rings carry exactly 8.4 MB. Stores ride the separate
            # SWDGE (gpsimd) path; consts go at the scalar ring's head.
            ring = [nc.sync.dma_start, nc.scalar.dma_start]
            nxfer = 0  # consts ride the scalar ring; t6 leads on sync

            ct = cpool.tile([128, 8 + NKA, 2, 16], F8)
            nc.scalar.dma_start(ct, cv)
            c2t = cpool.tile([128, NBETA, 16, 8], F8)
            nc.gpsimd.dma_start(c2t, cv2)
            pyb = [ybpool.tile([8, 512], F32, name=f"pyb{a}") for a in range(4)]
            ats = {}

            # HAM warmup: the PE idles from ~3 us until the first tile lands
            # (~12 us), then pays the cold 1.2 GHz clock for its first ~3.4 us
            # of matmuls. Fill the idle window with dummy matmuls on the
            # consts tile (resident by ~7 us) so the clock gate releases
            # before real work starts. They write pyb[0][:, 0:16], which the
            # rowgroup accumulation's start=True clears afterwards.
            for d in range(40):
                nc.tensor.matmul(
                    pyb[0][:, 0:16],
                    ct[:, d % 8, :, 0:8],
                    ct[:, d % 8, :, :],
                    start=True,
                    stop=True,
                    perf_mode=DR,
                    skip_group_check=True,
                )

            def load(dst, src):
                nonlocal nxfer
                ring[nxfer % 2](dst, src)
                nxfer += 1

            def do_y1(v):
                py1 = y1pool.tile([8, 512], F32, tag="py1", name="py1")
                tslab = l1pool.tile([128, NKA, 2, 512], F8, tag="l1", bufs=2)
                load(tslab, tv[:, v])
                for k in range(NKA):
                    nc.tensor.matmul(
                        py1,
                        ct[:, 8 + k, :, 0:8],
                        tslab[:, k],
                        start=(k == 0),
                        stop=(k == NKA - 1),
                        perf_mode=DR,
                    )
                y1s = spool.tile([8, 512], F32, tag="y1s", bufs=2, name="y1s")
                nc.vector.tensor_copy(y1s, py1)
                nc.gpsimd.dma_start(y1t[:, v * 512 : (v + 1) * 512], y1s)

            def y2_mms(slab, sl, t):
                py2 = y2pool.tile([8, 512], F32, tag="py2", name="py2")
                for k in range(8):
                    nc.tensor.matmul(
                        py2,
                        ct[:, k, :, 0:8],
                        slab[:, sl, k],
                        start=(k == 0),
                        stop=(k == 7),
                        perf_mode=DR,
                    )
                y2s = spool.tile([8, 512], F32, tag="y2s", bufs=3, name="y2s")
                nc.vector.tensor_copy(y2s, py2)
                nc.gpsimd.dma_start(y2t[:, t * 512 : (t + 1) * 512], y2s)

            def do_y2_single(t):
                slab1 = l2pool.tile([128, 1, 8, 2, 512], F8, tag="l2s", bufs=2)
                load(slab1, lv[:, t : t + 1])
                y2_mms(slab1, 0, t)
                if t in TBETA:
                    # DVE block-transpose the native unit (8.7 us, runs under
                    # the stream); the rowgroup matmuls are emitted LATER so
                    # the PE never waits on the transpose
                    at = apool.tile([128, 16, 512], F8, tag="at")
                    nc.vector.transpose(
                        at[:, :, :].rearrange("p c n -> p (c n)"),
                        slab1[:, 0, :, :, :].rearrange("p k m n -> p (k m n)"),
                    )
                    ats[t] = at

            def rowgroup_mms(t):
                # Y1 contraction over stripe t's 512 j values from the
                # block-transposed tile: A'[32a+u, cc, 32g+v] =
                # L[r0 + 128cc + 32a + v, 512t + 32g + u]
                tb = TBETA.index(t)
                av = ats[t][:, :, :].rearrange("p c (g v) -> p c g v", v=32)
                for b in range(16):
                    for a in range(4):
                        nc.tensor.matmul(
                            pyb[a],
                            c2t[32 * a : 32 * a + 32, tb, b, :],
                            av[32 * a : 32 * a + 32, :, b, :],
                            start=(tb == 0 and b == 0),
                            stop=(tb == NBETA - 1 and b == 15),
                            tile_position=(32 * a, 0),
                        )

            def do_y2_pair(t0):
                slab = l2pool.tile([128, 2, 8, 2, 512], F8, tag="l2p", bufs=2)
                load(slab, lv[:, t0 : t0 + 2])
                y2_mms(slab, 0, t0)
                y2_mms(slab, 1, t0 + 1)

            # stream order (ring alternation: consts/a, t6/s, t7/a, v0/s,
            # v1/a, t01/s, v2/a, t23/s, v3/a, t4/s, t5/a):
            do_y2_single(6)
            do_y2_single(7)
            do_y1(0)
            do_y1(1)
            rowgroup_mms(6)
            do_y2_pair(0)
            do_y1(2)
            rowgroup_mms(7)

            # drain the rowgroup accumulators immediately after their stop:
            # emitted here so the copies sit mid-queue on the Vector engine
            # (which is idle at this point) instead of running as pure tail
            # after the last stripe. Slab row i = 128*cc + 32a + v lives at
            # pyb[a][c, 32cc + v]. No loads in this block, so the ring
            # alternation of the remaining transfers is unchanged.
            bstage = spool.tile([128, 512], F32, tag="bst", bufs=1, name="bst")
            for a in range(4):
                nc.vector.tensor_copy(bstage[32 * a : 32 * a + 8, :], pyb[a])
                nc.gpsimd.dma_start(
                    y1bt[8 * a : 8 * a + 8, :], bstage[32 * a : 32 * a + 8, :]
                )

            do_y2_pair(2)
            do_y1(3)
            do_y2_single(4)
            do_y2_single(5)
    return nc


def _legalize_waits(nc):
    """Walrus on this toolchain allows at most ONE sync-wait per instruction.

    Two rewrites, applied to the finished BIR:
      1. Drop same-engine waits — every engine queue executes (and completes
         compute instructions) in order, so a wait on the engine's own
         semaphore from within its own stream is implied by program order.
      2. If an instruction still carries more than one wait, hoist all but
         the last onto fresh same-engine InstNoOps inserted just before it.
    """
    import concourse.mybir as mybir

    eng_prefix = {
        mybir.EngineType.PE: "PE_",
        mybir.EngineType.DVE: "DVE_",
        mybir.EngineType.Activation: "ACT_",
        mybir.EngineType.Pool: "Pool_",
        mybir.EngineType.SP: "SP_",
    }
    uid = 0
    for f in nc.m.functions:
        for b in f.blocks:
            out = []
            for inst in b.instructions:
                si = getattr(inst, "sync_info", None)
                waits = list(si.on_wait) if si is not None and si.on_wait else []
                if len(waits) > 1:
                    pref = eng_prefix.get(inst.engine)
                    if pref is not None:
                        keep = [
                            w
                            for w in waits
                            if not (w.ant_name or "").startswith(pref)
                        ]
                        waits = keep if keep else waits[-1:]
                    for w in waits[:-1]:
                        uid += 1
                        out.append(
                            mybir.InstNoOp(
                                name=f"lw-nop-{uid}",
                                engine=inst.engine,
                                sync_info=mybir.SyncInfo(
                                    on_wait=[w], on_update=[]
                                ),
                                bass_nofuse=True,
                            )
                        )
                    inst.sync_info = mybir.SyncInfo(
                        on_wait=waits[-1:],
                        on_update=list(si.on_update or []),
                    )
                out.append(inst)
            b.instructions[:] = out


def _get_nc():
    global _NC
    if _NC is None:
        nc = _build_program()
        _legalize_waits(nc)
        _NC = nc
    return _NC


# ------------------------------------------------------------- host math ---


def _sigmoid(x):
    x = np.asarray(x, np.float32)
    out = np.empty_like(x)
    pos = x >= 0
    out[pos] = 1.0 / (1.0 + np.exp(-x[pos]))
    ex = np.exp(x[~pos])
    out[~pos] = ex / (1.0 + ex)
    return out


def _softplus(x):
    x = np.asarray(x, np.float32)
    return np.log1p(np.exp(-np.abs(x))) + np.maximum(x, 0.0)


def _softmax(x, axis=-1):
    x = np.asarray(x, np.float32)
    m = np.max(x, axis=axis, keepdims=True)
    e = np.exp(x - m)
    return e / np.sum(e, axis=axis, keepdims=True)


def _content_weights(mem, keys, beta):
    # mem: [B,N,W], keys: [B,K,W], beta: [B,K] -> [B,K,N]
    dot = np.einsum("bnw,bkw->bkn", mem, keys, dtype=np.float32)
    mem_n = np.linalg.norm(mem, axis=-1)[:, None, :].astype(np.float32)
    key_n = np.linalg.norm(keys, axis=-1)[:, :, None].astype(np.float32)
    sim = dot / (mem_n * key_n + EPS)
    return _softmax(beta[..., None] * sim, axis=-1)


def _allocation(usage):
    idx = np.argsort(usage, axis=-1, kind="stable")
    sorted_u = np.take_along_axis(usage, idx, axis=-1)
    cp = np.cumprod(sorted_u, axis=-1)
    excl = np.concatenate([np.ones_like(cp[:, :1]), cp[:, :-1]], axis=-1)
    alloc_sorted = ((1.0 - sorted_u) * excl).astype(np.float32)
    out = np.empty_like(alloc_sorted)
    np.put_along_axis(out, idx, alloc_sorted, axis=-1)
    return out


# ----------------------------------------------------------------- kernel ---


def kernel(
    memory,
    usage,
    link,
    precedence,
    read_w_prev,
    write_w_prev,
    write_key,
    write_strength_raw,
    erase_raw,
    write_vec,
    free_raw,
    alloc_gate_raw,
    write_gate_raw,
    read_keys,
    read_strengths_raw,
    read_modes_raw,
):
    global LAST_RESULT
    from concourse.bass_utils import run_bass_kernel_spmd

    f32 = np.float32
    memory = np.asarray(memory, f32)
    usage = np.asarray(usage, f32)
    link = np.asarray(link, f32)
    precedence = np.asarray(precedence, f32)
    read_w_prev = np.asarray(read_w_prev, f32)
    write_w_prev = np.asarray(write_w_prev, f32)
    write_key = np.asarray(write_key, f32)
    write_strength_raw = np.asarray(write_strength_raw, f32)
    erase_raw = np.asarray(erase_raw, f32)
    write_vec = np.asarray(write_vec, f32)
    free_raw = np.asarray(free_raw, f32)
    alloc_gate_raw = np.asarray(alloc_gate_raw, f32)
    write_gate_raw = np.asarray(write_gate_raw, f32)
    read_keys = np.asarray(read_keys, f32)
    read_strengths_raw = np.asarray(read_strengths_raw, f32)
    read_modes_raw = np.asarray(read_modes_raw, f32)

    # --- interface activations ---
    write_strength = 1.0 + _softplus(write_strength_raw)  # [B]
    read_strengths = 1.0 + _softplus(read_strengths_raw)  # [B,R]
    erase = _sigmoid(erase_raw)  # [B,W]
    free = _sigmoid(free_raw)  # [B,R]
    g_a = _sigmoid(alloc_gate_raw)[:, None]  # [B,1]
    g_w = _sigmoid(write_gate_raw)[:, None]  # [B,1]
    modes = _softmax(read_modes_raw, axis=-1)  # [B,R,3]

    # --- write content addressing ---
    c_w = _content_weights(memory, write_key[:, None, :], write_strength[:, None])[
        :, 0
    ]  # [B,N]

    # --- usage update + allocation ---
    retention = np.prod(
        1.0 - free[..., None] * read_w_prev, axis=1, dtype=f32
    )  # [B,N]
    usage_new = ((usage + write_w_prev - usage * write_w_prev) * retention).astype(f32)
    alloc = _allocation(usage_new)  # [B,N]

    # --- write weights, memory erase/write ---
    w_w = (g_w * (g_a * alloc + (1.0 - g_a) * c_w)).astype(f32)  # [B,N]
    memory_new = (
        memory * (1.0 - w_w[:, :, None] * erase[:, None, :])
        + w_w[:, :, None] * write_vec[:, None, :]
    ).astype(f32)  # [B,N,W]

    # --- device part: Y1 = L @ V, Y2 = L^T @ V (per batch, split in 2 slabs) ---
    # V = [rwp^T | (w*rwp)^T]  ->  [N, 8]
    V = np.concatenate(
        [
            read_w_prev.transpose(0, 2, 1),  # [B,N,R]
            (w_w[:, :, None] * read_w_prev.transpose(0, 2, 1)),
        ],
        axis=2,
    ).astype(f32)  # [B,N,8]

    # Device runs fp8 e4m3 with an exact power-of-2 prescale: values of link
    # and V are O(1/N), so x4096 recenters them into fp8's normal range. The
    # output scale (4096^2 = 2^24) divides out exactly.
    SCALE = 4096.0
    f8 = ml_dtypes.float8_e4m3
    V8 = np.clip(V * SCALE, -240.0, 240.0).astype(f8)
    link8 = (link * SCALE).astype(f8)

    in_maps = []
    for core in range(NCORES):
        b, h = divmod(core, 2)
        r0 = h * SLAB
        nat = link8[b, r0 : r0 + SLAB, :]  # [2048, 4096]
        # rows r0 + 256k + 128m + p, cols 512t + n -> [p, t, k, m, n]
        lm = np.ascontiguousarray(
            nat.reshape(8, 2, 128, NJT, 512)
            .transpose(2, 3, 0, 1, 4)
            .reshape(128, NJT * 8192)
        )
        tr = nat.T  # [4096, 2048]: [j, i-r0]
        trk = tr.reshape(16, 2, 128, NIB, 512)[list(KARR)]  # alpha j-chunks
        tm = np.ascontiguousarray(
            trk.transpose(2, 3, 0, 1, 4).reshape(128, NIB * NKA * 1024)
        )
        VS = V8[b, r0 : r0 + SLAB]  # [2048, 8]
        VF = V8[b]  # [4096, 8]
        cs = VS.reshape(8, 2, 128, 8).transpose(2, 0, 1, 3)  # [128, 8, 2, 8]
        cf = VF.reshape(16, 2, 128, 8)[list(KARR)].transpose(2, 0, 1, 3)
        cq = np.concatenate([cs, cf], axis=1)  # [128, 8+NKA, 2, 8]
        cpad = np.zeros((128, 8 + NKA, 2, 16), dtype=f8)
        cpad[:, :, :, 0:8] = cq
        c2 = np.tile(
            VF.reshape(NJT, 16, 32, 8)[list(TBETA)].transpose(2, 0, 1, 3),
            (4, 1, 1, 1),
        )  # [128, NBETA, 16, 8]
        consts = np.ascontiguousarray(
            np.concatenate([cpad.reshape(128, -1), c2.reshape(128, -1)], axis=1)
        )
        in_maps.append({"lmat": lm, "tmat": tm, "consts": consts})

    nc = _get_nc()
    res = run_bass_kernel_spmd(
        nc,
        in_maps,
        list(range(NCORES)),
        trace=bool(os.environ.get("DNC_TRACE")),
    )
    LAST_RESULT = res

    UNSCALE = np.float32(1.0 / (SCALE * SCALE))
    Y1 = np.empty((B, N, 8), f32)
    Y2 = np.zeros((B, N, 8), f32)
    for core in range(NCORES):
        b, h = divmod(core, 2)
        r0 = h * SLAB
        yb = res.results[core]["y1bt"].reshape(4, 8, 16, 32)
        ybi = yb.transpose(2, 0, 3, 1).reshape(SLAB, 8)
        Y1[b, r0 : r0 + SLAB] = (res.results[core]["y1t"].T + ybi) * UNSCALE
        Y2[b] += res.results[core]["y2t"].T * UNSCALE

    A = Y1[..., :R].transpose(0, 2, 1)  # [B,R,N] = (L @ rwp_r)_i
    Bm = Y1[..., R:].transpose(0, 2, 1)  # (L @ (w*rwp_r))_i
    C = Y2[..., :R].transpose(0, 2, 1)  # (L^T @ rwp_r)_i
    D = Y2[..., R:].transpose(0, 2, 1)  # (L^T @ (w*rwp_r))_i

    w = w_w[:, None, :]  # [B,1,N]
    p = precedence[:, None, :]  # [B,1,N]
    s = np.einsum("bn,brn->br", precedence, read_w_prev, dtype=f32)[..., None]
    t = np.einsum("bn,brn->br", w_w, read_w_prev, dtype=f32)[..., None]
    diag = (w * p * read_w_prev).astype(f32)  # [B,R,N]

    fwd_w = ((1.0 - w) * A - Bm + w * s - diag).astype(f32)
    bwd_w = ((1.0 - w) * C - D + p * t - diag).astype(f32)

    # --- read content addressing + combine ---
    c_r = _content_weights(memory_new, read_keys, read_strengths)  # [B,R,N]
    read_w = (
        modes[..., 0:1] * bwd_w + modes[..., 1:2] * c_r + modes[..., 2:3] * fwd_w
    ).astype(f32)
    read_vectors = np.einsum("brn,bnw->brw", read_w, memory_new, dtype=f32)
    return read_vectors.astype(f32)


# revision 31
# speedup vs baseline: 1.1109x; 1.1109x over previous
"""EnhancedDNCMemory forward step on 8 Trainium2 NeuronCores.

Strategy
--------
The only heavy tensor is the temporal link matrix ``link`` [B=4, N=4096,
N=4096]. Everything else is O(N) or O(N*W) and is computed on the host in
float32.

The reference computes (per batch)::

    link_new = (1 - w_i - w_j) * link + w_i * p_j          (diag zeroed)
    fwd[r]   = link_new   @ rwp[r]
    bwd[r]   = link_new^T @ rwp[r]

Expanding link_new, both contractions decompose into matvecs against the
*raw* link matrix with the 8 stacked vectors V = [rwp^T | (w*rwp)^T] (N x 8):

    fwd[r,i] = (1-w_i)*(L@rwp_r)_i - (L@(w*rwp_r))_i + w_i*(p.rwp_r) - w_i*p_i*rwp_r_i
    bwd[r,i] = (1-w_i)*(L^T@rwp_r)_i - (L^T@(w*rwp_r))_i + p_i*(w.rwp_r) - w_i*p_i*rwp_r_i

So the device only computes Y1 = L_slab @ V and Y2 = L_slab^T @ V_slab.
Sharding: 8 cores = 4 batches x 2 row-slabs of 2048 rows. Y1 slabs
concatenate; Y2 partials sum (both on host, they are [N, 8] per core).

Device kernel (fp8 + DoubleRow + partial on-chip transposition):
  - Everything streams in fp8 e4m3 with an exact x4096 power-of-2 host
    prescale (values are O(1/N)); f32 PSUM accumulation. The 2^24 output
    scale divides out exactly. End-to-end relative error ~2e-4 (quantization
    errors average out across the 4096-term sums).
  - Y2 = L^T V streams the native-layout slab (contraction over rows =
    partitions) through MatmulPerfMode.DoubleRow matmuls: both operands are
    fp8 tiles shaped [128, 2, F], so each PE column-cycle contracts 256
    elements -- 2x the bf16 ingestion rate.
  - Y1 = L V needs the transposed orientation. For the ALPHA stripes
    (j outside TBETA) the host uploads a pre-transposed fp8 copy, consumed
    by the same DoubleRow form. For the BETA stripes the device reuses the
    native tiles: one DVE 32x32 block-transpose per stripe (8.7 us each,
    hidden under the stream) plus 64 row-grouped K=32 matmuls per stripe
    (4 concurrent via tile_position), accumulated in 4 PSUM tiles with a
    strided i-map that the host reorders. This trades ~3.4 us of spare PE
    time per stripe for 1.05 MB of HBM traffic.
  - DMA: ~14.7 MB/core in ~2 MB transfers alternating whole transfers
    across both HWDGE rings (descriptor generation is ~2.6 us per dma_start
    regardless of size -- one ring alone cannot feed 16 SDMA engines), with
    Y1/Y2 work interleaved so tile arrivals match the PE's static order.
    Stores and the rowgroup consts ride the separate SWDGE (gpsimd) path.
  - Both HBM copies are host-preswizzled so every DMA reads contiguous
    per-partition lines.

Toolchain notes: walrus on this stack allows at most ONE sync-wait per
instruction -- _legalize_waits() drops redundant same-engine waits and hoists
the rest onto same-engine NoOps, and the Tile kernel-tail drain is split
into one Drain per outstanding semaphore.
"""

import os

import ml_dtypes
import numpy as np

B = 4
N = 4096
W = 64
R = 4
NCORES = 8
SLAB = N // 2  # rows per core
EPS = 1e-6

NJT = N // 512  # 8 column tiles (Y2)
NIB = SLAB // 512  # 4 output row blocks (Y1)
TBETA = (6, 7)  # stripes whose Y1 j-contraction runs on-chip (DVE + rowgroups)
NBETA = len(TBETA)
KARR = tuple(
    k for t in range(NJT) if t not in TBETA for k in (2 * t, 2 * t + 1)
)  # tmat j-chunks (alpha stripes)
NKA = len(KARR)

_NC = None
LAST_RESULT = None


# ----------------------------------------------------------------- device ---


def _build_program():
    import concourse.bass as bass
    import concourse.mybir as mybir
    from concourse.tile import TileContext

    F8 = mybir.dt.float8e4
    F32 = mybir.dt.float32
    DR = mybir.MatmulPerfMode.DoubleRow

    class SplitDrainTileContext(TileContext):
        """Split the kernel-tail drain: walrus caps sync-waits per inst at 1."""

        def _drain_and_barrier(self, tick_clock, wait_clock):
            from concourse.vector_clock import ScopedClock, VectorClock

            vec = list(tick_clock.global_clock)
            nz = [i for i, t in enumerate(vec) if t > 0]
            for proc in nz:
                pv = VectorClock(
                    [t if j == proc else 0 for j, t in enumerate(vec)]
                )
                d = self.nc.sync.drain()
                wait_clock.add_sem_waits(d.ins, ScopedClock({None: pv}))
            if not nz:
                d = self.nc.sync.drain()
                wait_clock.add_sem_waits(
                    d.ins, ScopedClock({None: tick_clock.global_clock})
                )
            self.nc.all_engine_barrier()
            assert self.sems is not None
            popped = self.nc._tile_sem_poison_stack.pop()
            assert popped is self._sem_poison
            self.nc.clear_and_free_semaphores(list(self.sems.allocated().values()))
            self.nc.all_engine_barrier()

    nc = bass.Bass()
    # native slab, preswizzled: [p, t*8192 + k*1024 + m*512 + n]
    #   = L[r0 + 256k + 128m + p, 512t + n]
    lmat = nc.dram_tensor("lmat", [128, NJT * 8192], F8, kind="ExternalInput")
    # transposed slab, ALPHA stripes only: [p, v, k', m, n]
    #   = L[r0 + 512v + n, 256*KARR[k'] + 128m + p]
    tmat = nc.dram_tensor("tmat", [128, NIB * NKA * 1024], F8, kind="ExternalInput")
    # DR V chunks [p, q, m, 16] (q<8: VS, q=8+k': VF[KARR[k']]) + rowgroup V
    # [p=32a+u, tb, b, c8] = VF[512*TBETA[tb] + 32b + u, c]
    # (c padded 8 -> 16: dual-fp8 LDWEIGHTS requires the Ko=2 interleave
    # step to be a multiple of 16 bytes -- walrus s3_lw_dual_fp8_restrictions)
    NC1 = (8 + NKA) * 32
    consts = nc.dram_tensor(
        "consts", [128, NC1 + NBETA * 128], F8, kind="ExternalInput"
    )
    y1t = nc.dram_tensor("y1t", [8, SLAB], F32, kind="ExternalOutput")
    y1bt = nc.dram_tensor("y1bt", [32, 512], F32, kind="ExternalOutput")
    y2t = nc.dram_tensor("y2t", [8, N], F32, kind="ExternalOutput")

    lv = lmat[:, :].rearrange("p (t k m n) -> p t k m n", t=NJT, k=8, m=2, n=512)
    tv = tmat[:, :].rearrange("p (v k m n) -> p v k m n", v=NIB, k=NKA, m=2, n=512)
    cv = consts[:, 0:NC1].rearrange("p (q m c) -> p q m c", m=2, c=16)
    cv2 = consts[:, NC1:].rearrange("p (tb b c) -> p tb b c", b=16, c=8)

    with SplitDrainTileContext(nc) as tc:
        with (
            tc.tile_pool(name="cpool", bufs=1) as cpool,
            tc.tile_pool(name="l2pool", bufs=3) as l2pool,
            tc.tile_pool(name="l1pool", bufs=3) as l1pool,
            tc.tile_pool(name="apool", bufs=2) as apool,
            tc.tile_pool(name="spool", bufs=3) as spool,
            tc.tile_pool(name="ybpool", bufs=1, space="PSUM") as ybpool,
            tc.tile_pool(name="y1pool", bufs=2, space="PSUM") as y1pool,
            tc.tile_pool(name="y2pool", bufs=2, space="PSUM") as y2pool,
        ):
            # HWDGE descriptor generation costs ~2.6 us per dma_start
            # (always ~128 partition descriptors, size-independent), so the
            # stream uses FEW LARGE transfers alternating whole transfers
            # across the two rings (SP via nc.sync, ACT via nc.scalar) for
            # parallel generation at ~420 GB/s aggregate. Y1 and Y2 work is
            # INTERLEAVED so the PE's static order matches the arrival
            # stream (no dead zones from pool recycling); the stream starts
            # and ends with 1 MiB singles (fast ramp, short PE tail) and
            # both rings carry exactly 8.4 MB. Stores ride the separate
            # SWDGE (gpsimd) path; consts go at the scalar ring's head.
            ring = [nc.sync.dma_start, nc.scalar.dma_start]
            nxfer = 0  # consts ride the scalar ring; t6 leads on sync

            ct = cpool.tile([128, 8 + NKA, 2, 16], F8)
            nc.scalar.dma_start(ct, cv)
            c2t = cpool.tile([128, NBETA, 16, 8], F8)
            nc.gpsimd.dma_start(c2t, cv2)
            pyb = [ybpool.tile([8, 512], F32, name=f"pyb{a}") for a in range(4)]
            ats = {}

            def load(dst, src):
                nonlocal nxfer
                ring[nxfer % 2](dst, src)
                nxfer += 1

            def do_y1(v):
                py1 = y1pool.tile([8, 512], F32, tag="py1", name="py1")
                tslab = l1pool.tile([128, NKA, 2, 512], F8, tag="l1", bufs=2)
                load(tslab, tv[:, v])
                for k in range(NKA):
                    nc.tensor.matmul(
                        py1,
                        ct[:, 8 + k, :, 0:8],
                        tslab[:, k],
                        start=(k == 0),
                        stop=(k == NKA - 1),
                        perf_mode=DR,
                    )
                y1s = spool.tile([8, 512], F32, tag="y1s", bufs=2, name="y1s")
                nc.vector.tensor_copy(y1s, py1)
                nc.gpsimd.dma_start(y1t[:, v * 512 : (v + 1) * 512], y1s)

            def y2_mms(slab, sl, t):
                py2 = y2pool.tile([8, 512], F32, tag="py2", name="py2")
                for k in range(8):
                    nc.tensor.matmul(
                        py2,
                        ct[:, k, :, 0:8],
                        slab[:, sl, k],
                        start=(k == 0),
                        stop=(k == 7),
                        perf_mode=DR,
                    )
                y2s = spool.tile([8, 512], F32, tag="y2s", bufs=3, name="y2s")
                nc.vector.tensor_copy(y2s, py2)
                nc.gpsimd.dma_start(y2t[:, t * 512 : (t + 1) * 512], y2s)

            def do_y2_single(t):
                slab1 = l2pool.tile([128, 1, 8, 2, 512], F8, tag="l2s", bufs=2)
                load(slab1, lv[:, t : t + 1])
                y2_mms(slab1, 0, t)
                if t in TBETA:
                    # DVE block-transpose the native unit (8.7 us, runs under
                    # the stream); the rowgroup matmuls are emitted LATER so
                    # the PE never waits on the transpose
                    at = apool.tile([128, 16, 512], F8, tag="at")
                    nc.vector.transpose(
                        at[:, :, :].rearrange("p c n -> p (c n)"),
                        slab1[:, 0, :, :, :].rearrange("p k m n -> p (k m n)"),
                    )
                    ats[t] = at

            def rowgroup_mms(t):
                # Y1 contraction over stripe t's 512 j values from the
                # block-transposed tile: A'[32a+u, cc, 32g+v] =
                # L[r0 + 128cc + 32a + v, 512t + 32g + u]
                tb = TBETA.index(t)
                av = ats[t][:, :, :].rearrange("p c (g v) -> p c g v", v=32)
                for b in range(16):
                    for a in range(4):
                        nc.tensor.matmul(
                            pyb[a],
                            c2t[32 * a : 32 * a + 32, tb, b, :],
                            av[32 * a : 32 * a + 32, :, b, :],
                            start=(tb == 0 and b == 0),
                            stop=(tb == NBETA - 1 and b == 15),
                            tile_position=(32 * a, 0),
                        )

            def do_y2_pair(t0):
                slab = l2pool.tile([128, 2, 8, 2, 512], F8, tag="l2p", bufs=2)
                load(slab, lv[:, t0 : t0 + 2])
                y2_mms(slab, 0, t0)
                y2_mms(slab, 1, t0 + 1)

            # stream order (ring alternation: consts/a, t6/s, t7/a, v0/s,
            # v1/a, t01/s, v2/a, t23/s, v3/a, t4/s, t5/a):
            do_y2_single(6)
            do_y2_single(7)
            do_y1(0)
            do_y1(1)
            rowgroup_mms(6)
            do_y2_pair(0)
            do_y1(2)
            rowgroup_mms(7)

            # drain the rowgroup accumulators immediately after their stop:
            # emitted here so the copies sit mid-queue on the Vector engine
            # (which is idle at this point) instead of running as pure tail
            # after the last stripe. Slab row i = 128*cc + 32a + v lives at
            # pyb[a][c, 32cc + v]. No loads in this block, so the ring
            # alternation of the remaining transfers is unchanged.
            bstage = spool.tile([128, 512], F32, tag="bst", bufs=1, name="bst")
            for a in range(4):
                nc.vector.tensor_copy(bstage[32 * a : 32 * a + 8, :], pyb[a])
                nc.gpsimd.dma_start(
                    y1bt[8 * a : 8 * a + 8, :], bstage[32 * a : 32 * a + 8, :]
                )

            do_y2_pair(2)
            do_y1(3)
            do_y2_single(4)
            do_y2_single(5)
    return nc


def _legalize_waits(nc):
    """Walrus on this toolchain allows at most ONE sync-wait per instruction.

    Two rewrites, applied to the finished BIR:
      1. Drop same-engine waits — every engine queue executes (and completes
         compute instructions) in order, so a wait on the engine's own
         semaphore from within its own stream is implied by program order.
      2. If an instruction still carries more than one wait, hoist all but
         the last onto fresh same-engine InstNoOps inserted just before it.
    """
    import concourse.mybir as mybir

    eng_prefix = {
        mybir.EngineType.PE: "PE_",
        mybir.EngineType.DVE: "DVE_",
        mybir.EngineType.Activation: "ACT_",
        mybir.EngineType.Pool: "Pool_",
        mybir.EngineType.SP: "SP_",
    }
    uid = 0
    for f in nc.m.functions:
        for b in f.blocks:
            out = []
            for inst in b.instructions:
                si = getattr(inst, "sync_info", None)
                waits = list(si.on_wait) if si is not None and si.on_wait else []
                if len(waits) > 1:
                    pref = eng_prefix.get(inst.engine)
                    if pref is not None:
                        keep = [
                            w
                            for w in waits
                            if not (w.ant_name or "").startswith(pref)
                        ]
                        waits = keep if keep else waits[-1:]
                    for w in waits[:-1]:
                        uid += 1
                        out.append(
                            mybir.InstNoOp(
                                name=f"lw-nop-{uid}",
                                engine=inst.engine,
                                sync_info=mybir.SyncInfo(
                                    on_wait=[w], on_update=[]
                                ),
                                bass_nofuse=True,
                            )
                        )
                    inst.sync_info = mybir.SyncInfo(
                        on_wait=waits[-1:],
                        on_update=list(si.on_update or []),
                    )
                out.append(inst)
            b.instructions[:] = out


def _get_nc():
    global _NC
    if _NC is None:
        nc = _build_program()
        _legalize_waits(nc)
        _NC = nc
    return _NC


# ------------------------------------------------------------- host math ---


def _sigmoid(x):
    x = np.asarray(x, np.float32)
    out = np.empty_like(x)
    pos = x >= 0
    out[pos] = 1.0 / (1.0 + np.exp(-x[pos]))
    ex = np.exp(x[~pos])
    out[~pos] = ex / (1.0 + ex)
    return out


def _softplus(x):
    x = np.asarray(x, np.float32)
    return np.log1p(np.exp(-np.abs(x))) + np.maximum(x, 0.0)


def _softmax(x, axis=-1):
    x = np.asarray(x, np.float32)
    m = np.max(x, axis=axis, keepdims=True)
    e = np.exp(x - m)
    return e / np.sum(e, axis=axis, keepdims=True)


def _content_weights(mem, keys, beta):
    # mem: [B,N,W], keys: [B,K,W], beta: [B,K] -> [B,K,N]
    dot = np.einsum("bnw,bkw->bkn", mem, keys, dtype=np.float32)
    mem_n = np.linalg.norm(mem, axis=-1)[:, None, :].astype(np.float32)
    key_n = np.linalg.norm(keys, axis=-1)[:, :, None].astype(np.float32)
    sim = dot / (mem_n * key_n + EPS)
    return _softmax(beta[..., None] * sim, axis=-1)


def _allocation(usage):
    idx = np.argsort(usage, axis=-1, kind="stable")
    sorted_u = np.take_along_axis(usage, idx, axis=-1)
    cp = np.cumprod(sorted_u, axis=-1)
    excl = np.concatenate([np.ones_like(cp[:, :1]), cp[:, :-1]], axis=-1)
    alloc_sorted = ((1.0 - sorted_u) * excl).astype(np.float32)
    out = np.empty_like(alloc_sorted)
    np.put_along_axis(out, idx, alloc_sorted, axis=-1)
    return out


# ----------------------------------------------------------------- kernel ---


def kernel(
    memory,
    usage,
    link,
    precedence,
    read_w_prev,
    write_w_prev,
    write_key,
    write_strength_raw,
    erase_raw,
    write_vec,
    free_raw,
    alloc_gate_raw,
    write_gate_raw,
    read_keys,
    read_strengths_raw,
    read_modes_raw,
):
    global LAST_RESULT
    from concourse.bass_utils import run_bass_kernel_spmd

    f32 = np.float32
    memory = np.asarray(memory, f32)
    usage = np.asarray(usage, f32)
    link = np.asarray(link, f32)
    precedence = np.asarray(precedence, f32)
    read_w_prev = np.asarray(read_w_prev, f32)
    write_w_prev = np.asarray(write_w_prev, f32)
    write_key = np.asarray(write_key, f32)
    write_strength_raw = np.asarray(write_strength_raw, f32)
    erase_raw = np.asarray(erase_raw, f32)
    write_vec = np.asarray(write_vec, f32)
    free_raw = np.asarray(free_raw, f32)
    alloc_gate_raw = np.asarray(alloc_gate_raw, f32)
    write_gate_raw = np.asarray(write_gate_raw, f32)
    read_keys = np.asarray(read_keys, f32)
    read_strengths_raw = np.asarray(read_strengths_raw, f32)
    read_modes_raw = np.asarray(read_modes_raw, f32)

    # --- interface activations ---
    write_strength = 1.0 + _softplus(write_strength_raw)  # [B]
    read_strengths = 1.0 + _softplus(read_strengths_raw)  # [B,R]
    erase = _sigmoid(erase_raw)  # [B,W]
    free = _sigmoid(free_raw)  # [B,R]
    g_a = _sigmoid(alloc_gate_raw)[:, None]  # [B,1]
    g_w = _sigmoid(write_gate_raw)[:, None]  # [B,1]
    modes = _softmax(read_modes_raw, axis=-1)  # [B,R,3]

    # --- write content addressing ---
    c_w = _content_weights(memory, write_key[:, None, :], write_strength[:, None])[
        :, 0
    ]  # [B,N]

    # --- usage update + allocation ---
    retention = np.prod(
        1.0 - free[..., None] * read_w_prev, axis=1, dtype=f32
    )  # [B,N]
    usage_new = ((usage + write_w_prev - usage * write_w_prev) * retention).astype(f32)
    alloc = _allocation(usage_new)  # [B,N]

    # --- write weights, memory erase/write ---
    w_w = (g_w * (g_a * alloc + (1.0 - g_a) * c_w)).astype(f32)  # [B,N]
    memory_new = (
        memory * (1.0 - w_w[:, :, None] * erase[:, None, :])
        + w_w[:, :, None] * write_vec[:, None, :]
    ).astype(f32)  # [B,N,W]

    # --- device part: Y1 = L @ V, Y2 = L^T @ V (per batch, split in 2 slabs) ---
    # V = [rwp^T | (w*rwp)^T]  ->  [N, 8]
    V = np.concatenate(
        [
            read_w_prev.transpose(0, 2, 1),  # [B,N,R]
            (w_w[:, :, None] * read_w_prev.transpose(0, 2, 1)),
        ],
        axis=2,
    ).astype(f32)  # [B,N,8]

    # Device runs fp8 e4m3 with an exact power-of-2 prescale: values of link
    # and V are O(1/N), so x4096 recenters them into fp8's normal range. The
    # output scale (4096^2 = 2^24) divides out exactly.
    SCALE = 4096.0
    f8 = ml_dtypes.float8_e4m3
    V8 = np.clip(V * SCALE, -240.0, 240.0).astype(f8)
    link8 = (link * SCALE).astype(f8)

    in_maps = []
    for core in range(NCORES):
        b, h = divmod(core, 2)
        r0 = h * SLAB
        nat = link8[b, r0 : r0 + SLAB, :]  # [2048, 4096]
        # rows r0 + 256k + 128m + p, cols 512t + n -> [p, t, k, m, n]
        lm = np.ascontiguousarray(
            nat.reshape(8, 2, 128, NJT, 512)
            .transpose(2, 3, 0, 1, 4)
            .reshape(128, NJT * 8192)
        )
        tr = nat.T  # [4096, 2048]: [j, i-r0]
        trk = tr.reshape(16, 2, 128, NIB, 512)[list(KARR)]  # alpha j-chunks
        tm = np.ascontiguousarray(
            trk.transpose(2, 3, 0, 1, 4).reshape(128, NIB * NKA * 1024)
        )
        VS = V8[b, r0 : r0 + SLAB]  # [2048, 8]
        VF = V8[b]  # [4096, 8]
        cs = VS.reshape(8, 2, 128, 8).transpose(2, 0, 1, 3)  # [128, 8, 2, 8]
        cf = VF.reshape(16, 2, 128, 8)[list(KARR)].transpose(2, 0, 1, 3)
        cq = np.concatenate([cs, cf], axis=1)  # [128, 8+NKA, 2, 8]
        cpad = np.zeros((128, 8 + NKA, 2, 16), dtype=f8)
        cpad[:, :, :, 0:8] = cq
        c2 = np.tile(
            VF.reshape(NJT, 16, 32, 8)[list(TBETA)].transpose(2, 0, 1, 3),
            (4, 1, 1, 1),
        )  # [128, NBETA, 16, 8]
        consts = np.ascontiguousarray(
            np.concatenate([cpad.reshape(128, -1), c2.reshape(128, -1)], axis=1)
        )
        in_maps.append({"lmat": lm, "tmat": tm, "consts": consts})

    nc = _get_nc()
    res = run_bass_kernel_spmd(
        nc,
        in_maps,
        list(range(NCORES)),
        trace=bool(os.environ.get("DNC_TRACE")),
    )
    LAST_RESULT = res

    UNSCALE = np.float32(1.0 / (SCALE * SCALE))
    Y1 = np.empty((B, N, 8), f32)
    Y2 = np.zeros((B, N, 8), f32)
    for core in range(NCORES):
        b, h = divmod(core, 2)
        r0 = h * SLAB
        yb = res.results[core]["y1bt"].reshape(4, 8, 16, 32)
        ybi = yb.transpose(2, 0, 3, 1).reshape(SLAB, 8)
        Y1[b, r0 : r0 + SLAB] = (res.results[core]["y1t"].T + ybi) * UNSCALE
        Y2[b] += res.results[core]["y2t"].T * UNSCALE

    A = Y1[..., :R].transpose(0, 2, 1)  # [B,R,N] = (L @ rwp_r)_i
    Bm = Y1[..., R:].transpose(0, 2, 1)  # (L @ (w*rwp_r))_i
    C = Y2[..., :R].transpose(0, 2, 1)  # (L^T @ rwp_r)_i
    D = Y2[..., R:].transpose(0, 2, 1)  # (L^T @ (w*rwp_r))_i

    w = w_w[:, None, :]  # [B,1,N]
    p = precedence[:, None, :]  # [B,1,N]
    s = np.einsum("bn,brn->br", precedence, read_w_prev, dtype=f32)[..., None]
    t = np.einsum("bn,brn->br", w_w, read_w_prev, dtype=f32)[..., None]
    diag = (w * p * read_w_prev).astype(f32)  # [B,R,N]

    fwd_w = ((1.0 - w) * A - Bm + w * s - diag).astype(f32)
    bwd_w = ((1.0 - w) * C - D + p * t - diag).astype(f32)

    # --- read content addressing + combine ---
    c_r = _content_weights(memory_new, read_keys, read_strengths)  # [B,R,N]
    read_w = (
        modes[..., 0:1] * bwd_w + modes[..., 1:2] * c_r + modes[..., 2:3] * fwd_w
    ).astype(f32)
    read_vectors = np.einsum("brn,bnw->brw", read_w, memory_new, dtype=f32)
    return read_vectors.astype(f32)
